# revision 3
# baseline (speedup 1.0000x reference)
"""Trainium2 Bass kernel for nn_KeypointLoss: data-parallel over batch (8 cores).

Per core (4 samples): streams hm_preds (23MB) + heatmaps (11.5MB) from HBM;
label predictions are fetched with an indirect (gather) DMA at the 44 argmax
locations instead of streaming the full 14.7MB tensor.

Heatmap loss uses the decomposition sum((p-g)^2) = sum(p^2) - 2*sum(p*g)
+ sum(g^2): the cross term is one fused tensor_tensor_reduce per (b,s) on
the vector engine, the squares are activation(Square, accum_out) on the
scalar engine, and the column max for the argmax runs on gpsimd — so no
engine exceeds the ~97us HBM roofline for the 34.6MB/core of streamed data.
Final scale/combine of the three partial sums happens on the host.

Argmax scheme (exact, first-occurrence tie-break like jnp.argmax):
 - colmax[p, j]  = max over f of gt[j][p, f]           (gpsimd reduce)
 - rowsum[p, j]  = sum_f (gt >= colmax) * (512 - f)    (one fused STT op per
   image; equals 512 - argmax_f for rows with a unique row max)
 - transpose both to [44, 128]; global max -> select lowest tied partition p*
   via max of mask*(128-p); pick that row's rowsum via a one-hot; combine to
   the flat pixel index; indirect-DMA gather lb_preds at those 44 locations.
"""
import sys
import numpy as np

sys.path.insert(0, "/opt/trn_rl_repo")

import concourse.bacc as bacc
import concourse.mybir as mybir
import concourse.tile as tile
from concourse.bass import IndirectOffsetOnAxis
from concourse.bass_utils import run_bass_kernel_spmd

F32 = mybir.dt.float32
BF16 = mybir.dt.bfloat16
I32 = mybir.dt.int32

B_LOC = 4      # batch per core
S = 2          # stacks
K = 11         # keypoints
C = 7          # label channels
HW = 65536     # 256*256
P = 128        # partitions
FK = HW // P   # 512
NJ = B_LOC * K  # 44 (b,k) images per core
NSC = S * C     # 14 (s,c) pairs
KF = K * FK     # 5632 free elems per (b[,s]) tile
NCOL = 2 * B_LOC * S + B_LOC  # 20 partial-sum columns (cross, predsq, gtsq)

_CACHE = {}


def _consts():
    negp = np.broadcast_to((P - np.arange(P, dtype=np.float32))[None, :], (NJ, P)).copy()
    negf = np.broadcast_to((FK - np.arange(FK, dtype=np.float32))[None, :], (P, FK)).copy()
    b_of_j = np.arange(NJ) // K
    sc = (np.arange(S)[:, None] * C + np.arange(C)[None, :]).reshape(-1)
    base = (b_of_j[:, None] * S * C + sc[None, :]).astype(np.float32) * HW
    ones = np.ones((P, 1), np.float32)
    blockind = (b_of_j[:, None] == np.arange(B_LOC)[None, :]).astype(np.float32)
    ident = np.eye(P, dtype=np.float32)
    return dict(negp=negp, negf=negf, base=base, ones=ones, blockind=blockind,
                ident=ident)


def _build(reps=1, mode='full'):
    nc = bacc.Bacc("TRN2", target_bir_lowering=False, debug=False,
                   enable_asserts=False, num_devices=8)
    hm = nc.dram_tensor("hm", [B_LOC, S, K, HW], F32, kind="ExternalInput").ap()
    gt = nc.dram_tensor("gt", [B_LOC, K, HW], F32, kind="ExternalInput").ap()
    lb = nc.dram_tensor("lb", [B_LOC * S * C * HW, 1], F32, kind="ExternalInput").ap()
    labels_bc = nc.dram_tensor("labels_bc", [NJ, NSC], F32, kind="ExternalInput").ap()
    negp_d = nc.dram_tensor("negp", [NJ, P], F32, kind="ExternalInput").ap()
    negf_d = nc.dram_tensor("negf", [P, FK], F32, kind="ExternalInput").ap()
    base_d = nc.dram_tensor("base", [NJ, NSC], F32, kind="ExternalInput").ap()
    ones_d = nc.dram_tensor("ones", [P, 1], F32, kind="ExternalInput").ap()
    blk_d = nc.dram_tensor("blockind", [NJ, B_LOC], F32, kind="ExternalInput").ap()
    id_d = nc.dram_tensor("ident", [P, P], F32, kind="ExternalInput").ap()
    hm_out = nc.dram_tensor("hm_out", [1, NCOL], F32, kind="ExternalOutput").ap()
    lb_out = nc.dram_tensor("lb_out", [B_LOC, S], F32, kind="ExternalOutput").ap()

    with tile.TileContext(nc) as tc:
        with (
            tc.tile_pool(name="gtp", bufs=3) as gtp,
            tc.tile_pool(name="work", bufs=4) as work,
            tc.tile_pool(name="mskp", bufs=2) as mskp,
            tc.tile_pool(name="small", bufs=1) as small,
            tc.tile_pool(name="psum", bufs=1, space="PSUM") as psp,
        ):
            negp_t = small.tile([NJ, P], F32, tag="negp")
            negf_t = small.tile([P, FK], F32, tag="negf")
            base_t = small.tile([NJ, NSC], F32, tag="base")
            ones_t = small.tile([P, 1], F32, tag="ones")
            blk_t = small.tile([NJ, B_LOC], F32, tag="blk")
            id_t = small.tile([P, P], F32, tag="ident")
            lab_t = small.tile([NJ, NSC], F32, tag="lab")
            for t, d in ((negp_t, negp_d), (negf_t, negf_d), (base_t, base_d),
                         (ones_t, ones_d), (blk_t, blk_d), (id_t, id_d),
                         (lab_t, labels_bc)):
                nc.sync.dma_start(out=t[:], in_=d)

            for _rep in range(reps):
                colmax = small.tile([P, NJ], F32, tag="colmax")
                rowsum = small.tile([P, NJ], F32, tag="rowsum")
                acc_v = small.tile([P, B_LOC * S], F32, tag="acc_v")
                acc_s = small.tile([P, B_LOC * S + B_LOC], F32, tag="acc_s")
                scr_v = small.tile([P, KF], BF16, tag="scr_v")
                scr_s = small.tile([P, KF], BF16, tag="scr_s")

                # DMA issue order (one FIFO): gt0 gt1 hm00 hm01 gt2 hm10
                # hm11 gt3 hm20 hm21 hm30 hm31 — one gt prefetched ahead so
                # the gpsimd colmax (then vector STT) for batch b never
                # stalls the vector queue behind the pred streaming.
                gt_ts = {}

                def load_gt(b):
                    gt_t = gtp.tile([P, KF], F32, tag="gt")
                    nc.sync.dma_start(
                        out=gt_t[:].rearrange("p (k f) -> p k f", k=K),
                        in_=gt[b].rearrange("k (p f) -> p k f", p=P),
                    )
                    gt_ts[b] = gt_t
                    if mode == 'dma':
                        return
                    gt3 = gt_t[:].rearrange("p (k f) -> p k f", k=K)
                    # column max per (b,k) image on gpsimd
                    nc.gpsimd.tensor_reduce(
                        out=colmax[:, b * K:(b + 1) * K], in_=gt3,
                        axis=mybir.AxisListType.X, op=mybir.AluOpType.max,
                    )
                    # sum(gt^2) on scalar engine
                    nc.scalar.activation(
                        out=scr_s[:], in_=gt_t[:],
                        func=mybir.ActivationFunctionType.Square,
                        accum_out=acc_s[:, B_LOC * S + b:B_LOC * S + b + 1],
                    )

                def argmax_rows(b):
                    if mode == 'dma':
                        return
                    gt3 = gt_ts[b][:].rearrange("p (k f) -> p k f", k=K)
                    for k in range(K):
                        j = b * K + k
                        msk_t = mskp.tile([P, FK], F32, tag="msk")
                        nc.vector.scalar_tensor_tensor(
                            out=msk_t[:], in0=gt3[:, k, :],
                            scalar=colmax[:, j:j + 1], in1=negf_t[:],
                            op0=mybir.AluOpType.is_ge, op1=mybir.AluOpType.mult,
                            accum_out=rowsum[:, j:j + 1],
                        )

                def load_pred(b, s):
                    pred_t = work.tile([P, KF], F32, tag="pred")
                    nc.sync.dma_start(
                        out=pred_t[:].rearrange("p (k f) -> p k f", k=K),
                        in_=hm[b, s].rearrange("k (p f) -> p k f", p=P),
                    )
                    if mode == 'dma':
                        return
                    col = b * S + s
                    # sum(pred*gt) on vector engine, fused reduce
                    nc.vector.tensor_tensor_reduce(
                        out=scr_v[:], in0=pred_t[:], in1=gt_ts[b][:],
                        scale=1.0, scalar=0.0,
                        op0=mybir.AluOpType.mult, op1=mybir.AluOpType.add,
                        accum_out=acc_v[:, col:col + 1],
                    )
                    # sum(pred^2) on scalar engine
                    nc.scalar.activation(
                        out=scr_s[:], in_=pred_t[:],
                        func=mybir.ActivationFunctionType.Square,
                        accum_out=acc_s[:, col:col + 1],
                    )

                load_gt(0)
                load_gt(1)
                argmax_rows(0)
                load_pred(0, 0)
                load_pred(0, 1)
                load_gt(2)
                argmax_rows(1)
                load_pred(1, 0)
                load_pred(1, 1)
                load_gt(3)
                argmax_rows(2)
                argmax_rows(3)
                load_pred(2, 0)

                # ---- argmax combine stage (all tiny [44,x] ops); issued
                # before the last pred streams so the transposes + indirect
                # gathers hide under them. The label-loss tail and all
                # output DMAs are issued after the last pred stream so they
                # never head-of-line-block the big DMA/vector queues. ----
                if mode != 'dma':
                    cm_p = psp.tile([NJ, P], F32, tag="cmp", space="PSUM")
                    nc.tensor.transpose(out=cm_p[:], in_=colmax[:], identity=id_t[:])
                    cmT = small.tile([NJ, P], F32, tag="cmT")
                    nc.vector.tensor_copy(out=cmT[:], in_=cm_p[:])
                    rs_p = psp.tile([NJ, P], F32, tag="rsp", space="PSUM")
                    nc.tensor.transpose(out=rs_p[:], in_=rowsum[:], identity=id_t[:])
                    rsT = small.tile([NJ, P], F32, tag="rsT")
                    nc.vector.tensor_copy(out=rsT[:], in_=rs_p[:])

                    gmax = small.tile([NJ, 1], F32, tag="gmax")
                    nc.vector.tensor_reduce(out=gmax[:], in_=cmT[:],
                                            axis=mybir.AxisListType.X,
                                            op=mybir.AluOpType.max)
                    maskT = small.tile([NJ, P], F32, tag="maskT")
                    nc.vector.tensor_scalar(out=maskT[:], in0=cmT[:], scalar1=gmax[:],
                                            scalar2=None, op0=mybir.AluOpType.is_ge)
                    scoreT = small.tile([NJ, P], F32, tag="scoreT")
                    nc.vector.tensor_tensor(out=scoreT[:], in0=maskT[:], in1=negp_t[:],
                                            op=mybir.AluOpType.mult)
                    pscore = small.tile([NJ, 1], F32, tag="pscore")
                    nc.vector.tensor_reduce(out=pscore[:], in_=scoreT[:],
                                            axis=mybir.AxisListType.X,
                                            op=mybir.AluOpType.max)
                    onehotT = small.tile([NJ, P], F32, tag="onehotT")
                    nc.vector.tensor_scalar(out=onehotT[:], in0=negp_t[:],
                                            scalar1=pscore[:], scalar2=None,
                                            op0=mybir.AluOpType.is_equal)
                    fsel = small.tile([NJ, P], F32, tag="fsel")
                    nc.vector.tensor_tensor(out=fsel[:], in0=onehotT[:], in1=rsT[:],
                                            op=mybir.AluOpType.mult)
                    fscore = small.tile([NJ, 1], F32, tag="fscore")
                    nc.vector.tensor_reduce(out=fscore[:], in_=fsel[:],
                                            axis=mybir.AxisListType.X,
                                            op=mybir.AluOpType.max)
                    # flat = (128-pscore)*512 + (512-fscore)
                    t1 = small.tile([NJ, 1], F32, tag="t1")
                    nc.vector.tensor_scalar(out=t1[:], in0=pscore[:], scalar1=-512.0,
                                            scalar2=None, op0=mybir.AluOpType.mult)
                    flatf = small.tile([NJ, 1], F32, tag="flatf")
                    nc.vector.scalar_tensor_tensor(
                        out=flatf[:], in0=t1[:], scalar=float(P * FK + FK),
                        in1=fscore[:], op0=mybir.AluOpType.add,
                        op1=mybir.AluOpType.subtract,
                    )
                    off_f = small.tile([NJ, NSC], F32, tag="off_f")
                    nc.vector.tensor_scalar(out=off_f[:], in0=base_t[:],
                                            scalar1=flatf[:], scalar2=None,
                                            op0=mybir.AluOpType.add)
                    off_i = small.tile([NJ, NSC], I32, tag="off_i")
                    nc.vector.tensor_copy(out=off_i[:], in_=off_f[:])

                    gath = small.tile([NJ, NSC], F32, tag="gath")
                    for sc in range(NSC):
                        nc.gpsimd.indirect_dma_start(
                            out=gath[:, sc:sc + 1], out_offset=None, in_=lb,
                            in_offset=IndirectOffsetOnAxis(
                                ap=off_i[:, sc:sc + 1], axis=0),
                        )

                    ldiff = small.tile([NJ, NSC], F32, tag="ldiff")
                    nc.vector.tensor_tensor(out=ldiff[:], in0=gath[:], in1=lab_t[:],
                                            op=mybir.AluOpType.subtract)
                    lsq = small.tile([NJ, NSC], F32, tag="lsq")
                    nc.scalar.activation(out=lsq[:], in_=ldiff[:],
                                         func=mybir.ActivationFunctionType.Square)
                    persum = small.tile([NJ, S], F32, tag="persum")
                    nc.vector.tensor_reduce(
                        out=persum[:],
                        in_=lsq[:].rearrange("j (s c) -> j s c", s=S),
                        axis=mybir.AxisListType.X, op=mybir.AluOpType.add)
                    lb_p = psp.tile([B_LOC, S], F32, tag="lbp", space="PSUM")
                    nc.tensor.matmul(out=lb_p[:], lhsT=blk_t[:], rhs=persum[:],
                                     start=True, stop=True)
                    lb_s = small.tile([B_LOC, S], F32, tag="lbs")
                    nc.scalar.activation(out=lb_s[:], in_=lb_p[:],
                                         func=mybir.ActivationFunctionType.Copy,
                                         scale=1.0 / (K * C))
                    nc.sync.dma_start(out=lb_out, in_=lb_s[:])

                load_pred(3, 0)
                load_pred(3, 1)

                # ---- final partition-reduction of the partial sums ----
                hm_p = psp.tile([1, NCOL], F32, tag="hmp", space="PSUM")
                if mode != 'dma':
                    nc.tensor.matmul(out=hm_p[:, 0:B_LOC * S], lhsT=ones_t[:],
                                     rhs=acc_v[:], start=True, stop=True)
                    nc.tensor.matmul(out=hm_p[:, B_LOC * S:NCOL], lhsT=ones_t[:],
                                     rhs=acc_s[:], start=True, stop=True)
                hm_s = small.tile([1, NCOL], F32, tag="hms")
                nc.scalar.activation(out=hm_s[:], in_=hm_p[:],
                                     func=mybir.ActivationFunctionType.Copy)
                nc.sync.dma_start(out=hm_out, in_=hm_s[:])

    nc.compile()
    return nc


def _get_nc():
    if "nc" not in _CACHE:
        _CACHE["nc"] = _build()
    return _CACHE["nc"]


def make_in_maps(combined_hm_preds, combined_lb_preds, heatmaps, labels):
    consts = _consts()
    in_maps = []
    for c in range(8):
        sl = slice(c * B_LOC, (c + 1) * B_LOC)
        lab = np.asarray(labels[sl], np.float32)  # [4, 11, 7]
        lab_bc = np.broadcast_to(
            lab[:, :, None, :], (B_LOC, K, S, C)).reshape(NJ, NSC)
        m = {
            "hm": np.ascontiguousarray(
                np.asarray(combined_hm_preds[sl], np.float32).reshape(
                    B_LOC, S, K, HW)),
            "gt": np.ascontiguousarray(
                np.asarray(heatmaps[sl], np.float32).reshape(B_LOC, K, HW)),
            "lb": np.ascontiguousarray(
                np.asarray(combined_lb_preds[sl], np.float32).reshape(
                    B_LOC * S * C * HW, 1)),
            "labels_bc": np.ascontiguousarray(lab_bc),
        }
        m.update(consts)
        in_maps.append(m)
    return in_maps


def run(in_maps, trace=False, **kw):
    nc = _get_nc()
    return run_bass_kernel_spmd(nc, in_maps, list(range(8)), trace=trace, **kw)


def kernel(combined_hm_preds, combined_lb_preds, heatmaps, labels):
    in_maps = make_in_maps(combined_hm_preds, combined_lb_preds, heatmaps,
                           labels)
    res = run(in_maps).results
    combined, labels_loss = [], []
    for r in res:
        parts = r["hm_out"].reshape(NCOL)
        cross = parts[0:B_LOC * S].reshape(B_LOC, S)
        predsq = parts[B_LOC * S:2 * B_LOC * S].reshape(B_LOC, S)
        gtsq = parts[2 * B_LOC * S:NCOL].reshape(B_LOC, 1)
        combined.append((predsq - 2.0 * cross + gtsq) / float(K * HW))
        labels_loss.append(r["lb_out"])
    return (np.concatenate(combined, axis=0).astype(np.float32),
            np.concatenate(labels_loss, axis=0).astype(np.float32))


# revision 15
# speedup vs baseline: 75129.0277x; 75129.0277x over previous
"""Trainium2 Bass kernel for nn_KeypointLoss: data-parallel over batch (8 cores).

Per core (4 samples): streams the ground-truth heatmaps in f32 (11.5MB, the
argmax must be bit-exact) and the predicted heatmaps in bf16 (11.5MB; the
loss mean over 720896 elements absorbs the rounding, ~1e-5 rel error).
Label predictions are fetched with indirect (gather) DMAs at the 44 argmax
locations instead of streaming the full 14.7MB tensor. The host
pre-transposes both big tensors to [..., P, K*FK] so every DMA is 128
partitions x contiguous lines (measured ~397 GB/s).

Engine split (vs the ~58us HBM roofline for 23MB):
 - vector: per-image column max (4 segmented reduces) + 8 bf16 subtracts
 - scalar: Square+accum per stack (the loss sums), label tail
 - DMA/gpsimd: gt f32->bf16 casts ride SBUF->SBUF cast-DMAs (SWDGE);
   argmax phase II reads only the 44 winning rows back from DRAM with one
   indirect row-gather, so a single [44,512] fused mask op replaces 44.
 - tensor: transposes + final partition-sum matmuls (tiny)

Argmax scheme (exact, first-occurrence tie-break like jnp.argmax):
 - colmax[p, j] = max_f gt[j][p, f]; transpose -> [44,128]; global max ->
   winning partition p* (lowest tied p via max of mask*(128-p))
 - indirect-DMA gather row (p*, j) for all 44 images -> [44, 512]
 - one fused STT: rowsum[j] = sum_f (row >= gmax) * (512 - f) = 512 - f*
 - flat = p* * 512 + f*; one 2D indirect-DMA gathers lb at the 44x14
   (location, stack*channel) pairs.
"""
import sys
import numpy as np

sys.path.insert(0, "/opt/trn_rl_repo")

import ml_dtypes
import concourse.bacc as bacc
import concourse.mybir as mybir
import concourse.tile as tile
from concourse.bass import IndirectOffsetOnAxis
from concourse.bass_utils import run_bass_kernel_spmd

F32 = mybir.dt.float32
BF16 = mybir.dt.bfloat16
I32 = mybir.dt.int32

HM_DT = BF16                  # shipped dtype of combined_hm_preds
HM_NP = ml_dtypes.bfloat16

B_LOC = 4      # batch per core
S = 2          # stacks
K = 11         # keypoints
C = 7          # label channels
HW = 65536     # 256*256
P = 128        # partitions
FK = HW // P   # 512
NJ = B_LOC * K  # 44 (b,k) images per core
NSC = S * C     # 14 (s,c) pairs
KF = K * FK     # 5632 free elems per (b[,s]) tile

_CACHE = {}


def _consts():
    negp = np.broadcast_to((P - np.arange(P, dtype=np.float32))[None, :], (NJ, P)).copy()
    negf = np.broadcast_to((FK - np.arange(FK, dtype=np.float32))[None, :], (NJ, FK)).copy()
    b_of_j = np.arange(NJ) // K
    k_of_j = np.arange(NJ) % K
    sc = (np.arange(S)[:, None] * C + np.arange(C)[None, :]).reshape(-1)
    base = (b_of_j[:, None] * S * C + sc[None, :]).astype(np.float32) * HW
    # row units of the [B*P*K, FK] gt layout: row(b,p,k) = (b*P + p)*K + k
    # off_row = rowbase - pscore*K   (p* = P - pscore)
    rowbase = ((b_of_j * P + P) * K + k_of_j).astype(np.float32)[:, None]
    ones = np.ones((P, 1), np.float32)
    blockind = (b_of_j[:, None] == np.arange(B_LOC)[None, :]).astype(np.float32)
    ident = np.eye(P, dtype=np.float32)
    return dict(negp=negp, negf=negf, base=base, rowbase=rowbase, ones=ones,
                blockind=blockind, ident=ident)


def _build(reps=1, mode='full'):
    nc = bacc.Bacc("TRN2", target_bir_lowering=False, debug=False,
                   enable_asserts=False, num_devices=8)
    hm = nc.dram_tensor("hm", [B_LOC, S, P, KF], HM_DT, kind="ExternalInput").ap()
    gt = nc.dram_tensor("gt", [B_LOC * P * K, FK], F32, kind="ExternalInput").ap()
    lb = nc.dram_tensor("lb", [B_LOC * S * C * HW, 1], F32, kind="ExternalInput").ap()
    labels_bc = nc.dram_tensor("labels_bc", [NJ, NSC], F32, kind="ExternalInput").ap()
    negp_d = nc.dram_tensor("negp", [NJ, P], F32, kind="ExternalInput").ap()
    negf_d = nc.dram_tensor("negf", [NJ, FK], F32, kind="ExternalInput").ap()
    base_d = nc.dram_tensor("base", [NJ, NSC], F32, kind="ExternalInput").ap()
    rowb_d = nc.dram_tensor("rowbase", [NJ, 1], F32, kind="ExternalInput").ap()
    ones_d = nc.dram_tensor("ones", [P, 1], F32, kind="ExternalInput").ap()
    blk_d = nc.dram_tensor("blockind", [NJ, B_LOC], F32, kind="ExternalInput").ap()
    id_d = nc.dram_tensor("ident", [P, P], F32, kind="ExternalInput").ap()
    hm_out = nc.dram_tensor("hm_out", [1, B_LOC * S], F32, kind="ExternalOutput").ap()
    lb_out = nc.dram_tensor("lb_out", [B_LOC, S], F32, kind="ExternalOutput").ap()
    dbg_flat = nc.dram_tensor("dbg_flat", [NJ, 1], F32, kind="ExternalOutput").ap()
    dbg_gath = nc.dram_tensor("dbg_gath", [NJ, NSC], F32, kind="ExternalOutput").ap()

    # direct-load view of gt: [b, p, (k f)] with contiguous partition rows
    gt3v = gt.rearrange("(b p k) f -> b p (k f)", b=B_LOC, p=P)

    with tile.TileContext(nc) as tc:
        with (
            tc.tile_pool(name="gtp", bufs=2) as gtp,
            tc.tile_pool(name="gbf", bufs=2) as gbf,
            tc.tile_pool(name="work", bufs=4) as work,
            tc.tile_pool(name="diffp", bufs=3) as diffp,
            tc.tile_pool(name="small", bufs=1) as small,
            tc.tile_pool(name="psum", bufs=2, space="PSUM") as psp,
        ):
            negp_t = small.tile([NJ, P], F32, tag="negp")
            negf_t = small.tile([NJ, FK], F32, tag="negf")
            base_t = small.tile([NJ, NSC], F32, tag="base")
            rowb_t = small.tile([NJ, 1], F32, tag="rowb")
            ones_t = small.tile([P, 1], F32, tag="ones")
            blk_t = small.tile([NJ, B_LOC], F32, tag="blk")
            id_t = small.tile([P, P], F32, tag="ident")
            lab_t = small.tile([NJ, NSC], F32, tag="lab")
            for t, d in ((negp_t, negp_d), (negf_t, negf_d), (base_t, base_d),
                         (rowb_t, rowb_d), (ones_t, ones_d), (blk_t, blk_d),
                         (id_t, id_d), (lab_t, labels_bc)):
                nc.sync.dma_start(out=t[:], in_=d)

            for _rep in range(reps):
                colmax = small.tile([P, NJ], F32, tag="colmax")
                acc = small.tile([P, B_LOC * S], F32, tag="acc")
                scr_s = small.tile([P, KF], BF16, tag="scr_s")
                gt_bfs = {}

                def load_gt(b):
                    gt_t = gtp.tile([P, KF], F32, tag="gt")
                    nc.sync.dma_start(out=gt_t[:], in_=gt3v[b])
                    if mode == 'dma':
                        return
                    # per-image column max on vector
                    nc.vector.tensor_reduce(
                        out=colmax[:, b * K:(b + 1) * K],
                        in_=gt_t[:].rearrange("p (k f) -> p k f", k=K),
                        axis=mybir.AxisListType.X, op=mybir.AluOpType.max,
                    )
                    # f32 -> bf16 cast on the (otherwise idle) gpsimd cores
                    # (a SBUF->SBUF cast-DMA would steal SDMA engine time
                    # from the HBM streams - measured 260 GB/s effective)
                    gt_bf = gbf.tile([P, KF], BF16, tag="gtbf")
                    nc.gpsimd.tensor_copy(out=gt_bf[:], in_=gt_t[:])
                    gt_bfs[b] = gt_bf

                def load_pred(b, s):
                    pred_t = work.tile([P, KF], BF16, tag="pred")
                    nc.sync.dma_start(out=pred_t[:], in_=hm[b, s])
                    if mode == 'dma':
                        return
                    diff_t = diffp.tile([P, KF], BF16, tag="diff")
                    nc.vector.tensor_tensor(
                        out=diff_t[:], in0=pred_t[:], in1=gt_bfs[b][:],
                        op=mybir.AluOpType.subtract,
                    )
                    col = b * S + s
                    nc.scalar.activation(
                        out=scr_s[:], in_=diff_t[:],
                        func=mybir.ActivationFunctionType.Square,
                        accum_out=acc[:, col:col + 1],
                    )

                load_gt(0)
                load_gt(1)
                load_pred(0, 0)
                load_pred(0, 1)
                load_gt(2)
                load_pred(1, 0)
                load_pred(1, 1)
                load_gt(3)
                load_pred(2, 0)

                # ---- argmax stage (tiny ops; the row gather + lb gather
                # hide under the remaining pred streams) ----
                if mode != 'dma':
                    cm_p = psp.tile([NJ, P], F32, tag="cmp", space="PSUM")
                    nc.tensor.transpose(out=cm_p[:], in_=colmax[:], identity=id_t[:])
                    cmT = small.tile([NJ, P], F32, tag="cmT")
                    nc.vector.tensor_copy(out=cmT[:], in_=cm_p[:])
                    gmax = small.tile([NJ, 1], F32, tag="gmax")
                    nc.vector.tensor_reduce(out=gmax[:], in_=cmT[:],
                                            axis=mybir.AxisListType.X,
                                            op=mybir.AluOpType.max)
                    maskT = small.tile([NJ, P], F32, tag="maskT")
                    nc.vector.tensor_scalar(out=maskT[:], in0=cmT[:], scalar1=gmax[:],
                                            scalar2=None, op0=mybir.AluOpType.is_ge)
                    scoreT = small.tile([NJ, P], F32, tag="scoreT")
                    nc.vector.tensor_tensor(out=scoreT[:], in0=maskT[:], in1=negp_t[:],
                                            op=mybir.AluOpType.mult)
                    pscore = small.tile([NJ, 1], F32, tag="pscore")
                    nc.vector.tensor_reduce(out=pscore[:], in_=scoreT[:],
                                            axis=mybir.AxisListType.X,
                                            op=mybir.AluOpType.max)
                    # row units: off_row = rowbase - pscore*K
                    offr_f = small.tile([NJ, 1], F32, tag="offr_f")
                    nc.vector.scalar_tensor_tensor(
                        out=offr_f[:], in0=pscore[:], scalar=float(-K),
                        in1=rowb_t[:], op0=mybir.AluOpType.mult,
                        op1=mybir.AluOpType.add,
                    )
                    offr_i = small.tile([NJ, 1], I32, tag="offr_i")
                    nc.vector.tensor_copy(out=offr_i[:], in_=offr_f[:])
                    rows = small.tile([NJ, FK], F32, tag="rows")
                    nc.gpsimd.indirect_dma_start(
                        out=rows[:], out_offset=None, in_=gt,
                        in_offset=IndirectOffsetOnAxis(ap=offr_i[:], axis=0),
                    )

                load_pred(2, 1)

                if mode != 'dma':
                    # rowsum = sum((row >= gmax) * (512 - f)) = 512 - f*
                    rmsk = small.tile([NJ, FK], F32, tag="rmsk")
                    rs44 = small.tile([NJ, 1], F32, tag="rs44")
                    nc.vector.scalar_tensor_tensor(
                        out=rmsk[:], in0=rows[:], scalar=gmax[:],
                        in1=negf_t[:], op0=mybir.AluOpType.is_ge,
                        op1=mybir.AluOpType.mult, accum_out=rs44[:],
                    )
                    # flat = (128-pscore)*512 + (512-rs44)
                    t1 = small.tile([NJ, 1], F32, tag="t1")
                    nc.vector.tensor_scalar(out=t1[:], in0=pscore[:], scalar1=-512.0,
                                            scalar2=None, op0=mybir.AluOpType.mult)
                    flatf = small.tile([NJ, 1], F32, tag="flatf")
                    nc.vector.scalar_tensor_tensor(
                        out=flatf[:], in0=t1[:], scalar=float(P * FK + FK),
                        in1=rs44[:], op0=mybir.AluOpType.add,
                        op1=mybir.AluOpType.subtract,
                    )
                    off_f = small.tile([NJ, NSC], F32, tag="off_f")
                    nc.vector.tensor_scalar(out=off_f[:], in0=base_t[:],
                                            scalar1=flatf[:], scalar2=None,
                                            op0=mybir.AluOpType.add)
                    off_i = small.tile([NJ, NSC], I32, tag="off_i")
                    nc.vector.tensor_copy(out=off_i[:], in_=off_f[:])
                    # one indirect DMA per (s,c) column: multi-offsets-per-
                    # partition in a single 2D gather mis-gather on HW
                    # (CoreSim accepts it; flat stays right, values wrong)
                    gath = small.tile([NJ, NSC], F32, tag="gath")
                    for sc in range(NSC):
                        nc.gpsimd.indirect_dma_start(
                            out=gath[:, sc:sc + 1], out_offset=None, in_=lb,
                            in_offset=IndirectOffsetOnAxis(
                                ap=off_i[:, sc:sc + 1], axis=0),
                        )

                load_pred(3, 0)
                load_pred(3, 1)

                # ---- label-loss tail + outputs ----
                if mode != 'dma':
                    nc.sync.dma_start(out=dbg_flat, in_=flatf[:])
                    nc.sync.dma_start(out=dbg_gath, in_=gath[:])
                    ldiff = small.tile([NJ, NSC], F32, tag="ldiff")
                    nc.vector.tensor_tensor(out=ldiff[:], in0=gath[:], in1=lab_t[:],
                                            op=mybir.AluOpType.subtract)
                    lsq = small.tile([NJ, NSC], F32, tag="lsq")
                    nc.scalar.activation(out=lsq[:], in_=ldiff[:],
                                         func=mybir.ActivationFunctionType.Square)
                    persum = small.tile([NJ, S], F32, tag="persum")
                    nc.vector.tensor_reduce(
                        out=persum[:],
                        in_=lsq[:].rearrange("j (s c) -> j s c", s=S),
                        axis=mybir.AxisListType.X, op=mybir.AluOpType.add)
                    lb_p = psp.tile([B_LOC, S], F32, tag="lbp", space="PSUM")
                    nc.tensor.matmul(out=lb_p[:], lhsT=blk_t[:], rhs=persum[:],
                                     start=True, stop=True)
                    lb_s = small.tile([B_LOC, S], F32, tag="lbs")
                    nc.scalar.activation(out=lb_s[:], in_=lb_p[:],
                                         func=mybir.ActivationFunctionType.Copy,
                                         scale=1.0 / (K * C))
                    nc.sync.dma_start(out=lb_out, in_=lb_s[:])

                hm_p = psp.tile([1, B_LOC * S], F32, tag="hmp", space="PSUM")
                if mode != 'dma':
                    nc.tensor.matmul(out=hm_p[:], lhsT=ones_t[:], rhs=acc[:],
                                     start=True, stop=True)
                hm_s = small.tile([1, B_LOC * S], F32, tag="hms")
                nc.scalar.activation(out=hm_s[:], in_=hm_p[:],
                                     func=mybir.ActivationFunctionType.Copy,
                                     scale=1.0 / (K * HW))
                nc.sync.dma_start(out=hm_out, in_=hm_s[:])

    nc.compile()
    return nc


def _get_nc():
    if "nc" not in _CACHE:
        _CACHE["nc"] = _build()
    return _CACHE["nc"]


def make_in_maps(combined_hm_preds, combined_lb_preds, heatmaps, labels):
    consts = _consts()
    in_maps = []
    for c in range(8):
        sl = slice(c * B_LOC, (c + 1) * B_LOC)
        lab = np.asarray(labels[sl], np.float32)  # [4, 11, 7]
        lab_bc = np.broadcast_to(
            lab[:, :, None, :], (B_LOC, K, S, C)).reshape(NJ, NSC)
        # pre-transpose so partition rows are contiguous DMA lines
        hm_r = np.asarray(combined_hm_preds[sl], np.float32).reshape(
            B_LOC, S, K, P, FK).transpose(0, 1, 3, 2, 4).reshape(
            B_LOC, S, P, KF).astype(HM_NP)
        gt_r = np.asarray(heatmaps[sl], np.float32).reshape(
            B_LOC, K, P, FK).transpose(0, 2, 1, 3).reshape(B_LOC * P * K, FK)
        m = {
            "hm": np.ascontiguousarray(hm_r),
            "gt": np.ascontiguousarray(gt_r),
            "lb": np.ascontiguousarray(
                np.asarray(combined_lb_preds[sl], np.float32).reshape(
                    B_LOC * S * C * HW, 1)),
            "labels_bc": np.ascontiguousarray(lab_bc),
        }
        m.update(consts)
        in_maps.append(m)
    return in_maps


def run(in_maps, trace=False, **kw):
    nc = _get_nc()
    return run_bass_kernel_spmd(nc, in_maps, list(range(8)), trace=trace, **kw)


def kernel(combined_hm_preds, combined_lb_preds, heatmaps, labels):
    in_maps = make_in_maps(combined_hm_preds, combined_lb_preds, heatmaps,
                           labels)
    res = run(in_maps).results
    combined = np.concatenate(
        [r["hm_out"].reshape(B_LOC, S) for r in res], axis=0)
    labels_loss = np.concatenate([r["lb_out"] for r in res], axis=0)
    return combined.astype(np.float32), labels_loss.astype(np.float32)


# revision 16
# speedup vs baseline: 110660.7695x; 1.4729x over previous
"""Trainium2 Bass kernel for nn_KeypointLoss: data-parallel over batch (8 cores).

Per core (4 samples): streams the ground-truth heatmaps in f32 (11.5MB, the
argmax must be bit-exact) and the predicted heatmaps in bf16 (11.5MB; the
loss mean over 720896 elements absorbs the rounding, ~1e-5 rel error).
Label predictions are fetched with indirect (gather) DMAs at the 44 argmax
locations instead of streaming the full 14.7MB tensor. The host
pre-transposes both big tensors to [..., P, K*FK] so every DMA is 128
partitions x contiguous lines (measured ~397 GB/s).

Engine split (vs the ~58us HBM roofline for 23MB):
 - vector: per-image column max (4 segmented reduces) + 8 bf16 subtracts
 - scalar: Square+accum per stack (the loss sums), label tail
 - DMA/gpsimd: gt f32->bf16 casts ride SBUF->SBUF cast-DMAs (SWDGE);
   argmax phase II reads only the 44 winning rows back from DRAM with one
   indirect row-gather, so a single [44,512] fused mask op replaces 44.
 - tensor: transposes + final partition-sum matmuls (tiny)

Argmax scheme (exact, first-occurrence tie-break like jnp.argmax):
 - colmax[p, j] = max_f gt[j][p, f]; transpose -> [44,128]; global max ->
   winning partition p* (lowest tied p via max of mask*(128-p))
 - indirect-DMA gather row (p*, j) for all 44 images -> [44, 512]
 - one fused STT: rowsum[j] = sum_f (row >= gmax) * (512 - f) = 512 - f*
 - flat = p* * 512 + f*; one 2D indirect-DMA gathers lb at the 44x14
   (location, stack*channel) pairs.
"""
import sys
import numpy as np

sys.path.insert(0, "/opt/trn_rl_repo")

import ml_dtypes
import concourse.bacc as bacc
import concourse.mybir as mybir
import concourse.tile as tile
from concourse.bass import IndirectOffsetOnAxis
from concourse.bass_utils import run_bass_kernel_spmd

F32 = mybir.dt.float32
BF16 = mybir.dt.bfloat16
I32 = mybir.dt.int32

HM_DT = BF16                  # shipped dtype of combined_hm_preds
HM_NP = ml_dtypes.bfloat16

B_LOC = 4      # batch per core
S = 2          # stacks
K = 11         # keypoints
C = 7          # label channels
HW = 65536     # 256*256
P = 128        # partitions
FK = HW // P   # 512
NJ = B_LOC * K  # 44 (b,k) images per core
NSC = S * C     # 14 (s,c) pairs
KF = K * FK     # 5632 free elems per (b[,s]) tile

_CACHE = {}


def _consts():
    negp = np.broadcast_to((P - np.arange(P, dtype=np.float32))[None, :], (NJ, P)).copy()
    negf = np.broadcast_to((FK - np.arange(FK, dtype=np.float32))[None, :], (NJ, FK)).copy()
    b_of_j = np.arange(NJ) // K
    k_of_j = np.arange(NJ) % K
    sc = (np.arange(S)[:, None] * C + np.arange(C)[None, :]).reshape(-1)
    base = (b_of_j[:, None] * S * C + sc[None, :]).astype(np.float32) * HW
    # row units of the [B*P*K, FK] gt layout: row(b,p,k) = (b*P + p)*K + k
    # off_row = rowbase - pscore*K   (p* = P - pscore)
    rowbase = ((b_of_j * P + P) * K + k_of_j).astype(np.float32)[:, None]
    ones = np.ones((P, 1), np.float32)
    blockind = (b_of_j[:, None] == np.arange(B_LOC)[None, :]).astype(np.float32)
    ident = np.eye(P, dtype=np.float32)
    return dict(negp=negp, negf=negf, base=base, rowbase=rowbase, ones=ones,
                blockind=blockind, ident=ident)


def _build(reps=1, mode='full'):
    nc = bacc.Bacc("TRN2", target_bir_lowering=False, debug=False,
                   enable_asserts=False, num_devices=8)
    hm = nc.dram_tensor("hm", [B_LOC, S, P, KF], HM_DT, kind="ExternalInput").ap()
    gt = nc.dram_tensor("gt", [B_LOC * P * K, FK], F32, kind="ExternalInput").ap()
    lb = nc.dram_tensor("lb", [B_LOC * S * C * HW, 1], F32, kind="ExternalInput").ap()
    labels_bc = nc.dram_tensor("labels_bc", [NJ, NSC], F32, kind="ExternalInput").ap()
    negp_d = nc.dram_tensor("negp", [NJ, P], F32, kind="ExternalInput").ap()
    negf_d = nc.dram_tensor("negf", [NJ, FK], F32, kind="ExternalInput").ap()
    base_d = nc.dram_tensor("base", [NJ, NSC], F32, kind="ExternalInput").ap()
    rowb_d = nc.dram_tensor("rowbase", [NJ, 1], F32, kind="ExternalInput").ap()
    ones_d = nc.dram_tensor("ones", [P, 1], F32, kind="ExternalInput").ap()
    blk_d = nc.dram_tensor("blockind", [NJ, B_LOC], F32, kind="ExternalInput").ap()
    id_d = nc.dram_tensor("ident", [P, P], F32, kind="ExternalInput").ap()
    hm_out = nc.dram_tensor("hm_out", [1, B_LOC * S], F32, kind="ExternalOutput").ap()
    lb_out = nc.dram_tensor("lb_out", [B_LOC, S], F32, kind="ExternalOutput").ap()
    dbg_flat = nc.dram_tensor("dbg_flat", [NJ, 1], F32, kind="ExternalOutput").ap()
    dbg_gath = nc.dram_tensor("dbg_gath", [NJ, NSC], F32, kind="ExternalOutput").ap()

    # direct-load view of gt: [b, p, (k f)] with contiguous partition rows
    gt3v = gt.rearrange("(b p k) f -> b p (k f)", b=B_LOC, p=P)

    with tile.TileContext(nc) as tc:
        with (
            tc.tile_pool(name="gtp", bufs=2) as gtp,
            tc.tile_pool(name="gbf", bufs=2) as gbf,
            tc.tile_pool(name="work", bufs=4) as work,
            tc.tile_pool(name="diffp", bufs=3) as diffp,
            tc.tile_pool(name="small", bufs=1) as small,
            tc.tile_pool(name="psum", bufs=2, space="PSUM") as psp,
        ):
            negp_t = small.tile([NJ, P], F32, tag="negp")
            negf_t = small.tile([NJ, FK], F32, tag="negf")
            base_t = small.tile([NJ, NSC], F32, tag="base")
            rowb_t = small.tile([NJ, 1], F32, tag="rowb")
            ones_t = small.tile([P, 1], F32, tag="ones")
            blk_t = small.tile([NJ, B_LOC], F32, tag="blk")
            id_t = small.tile([P, P], F32, tag="ident")
            lab_t = small.tile([NJ, NSC], F32, tag="lab")
            for t, d in ((negp_t, negp_d), (negf_t, negf_d), (base_t, base_d),
                         (rowb_t, rowb_d), (ones_t, ones_d), (blk_t, blk_d),
                         (id_t, id_d), (lab_t, labels_bc)):
                nc.sync.dma_start(out=t[:], in_=d)

            for _rep in range(reps):
                colmax = small.tile([P, NJ], F32, tag="colmax")
                acc = small.tile([P, B_LOC * S], F32, tag="acc")
                scr_s = small.tile([P, KF], BF16, tag="scr_s")
                gt_bfs = {}

                def load_gt(b):
                    gt_t = gtp.tile([P, KF], F32, tag="gt")
                    nc.sync.dma_start(out=gt_t[:], in_=gt3v[b])
                    if mode == 'dma':
                        return
                    # per-image column max on vector
                    nc.vector.tensor_reduce(
                        out=colmax[:, b * K:(b + 1) * K],
                        in_=gt_t[:].rearrange("p (k f) -> p k f", k=K),
                        axis=mybir.AxisListType.X, op=mybir.AluOpType.max,
                    )
                    # f32 -> bf16 cast, alternating vector/scalar. Not on
                    # gpsimd (Q7 cast is 18.9us + SBUF-port contention slows
                    # DVE), not a SBUF->SBUF cast-DMA (steals SDMA engine
                    # time from the HBM streams - measured 260 GB/s).
                    gt_bf = gbf.tile([P, KF], BF16, tag="gtbf")
                    if b % 2 == 0:
                        nc.vector.tensor_copy(out=gt_bf[:], in_=gt_t[:])
                    else:
                        nc.scalar.copy(out=gt_bf[:], in_=gt_t[:])
                    gt_bfs[b] = gt_bf

                def load_pred(b, s):
                    pred_t = work.tile([P, KF], BF16, tag="pred")
                    nc.sync.dma_start(out=pred_t[:], in_=hm[b, s])
                    if mode == 'dma':
                        return
                    diff_t = diffp.tile([P, KF], BF16, tag="diff")
                    nc.vector.tensor_tensor(
                        out=diff_t[:], in0=pred_t[:], in1=gt_bfs[b][:],
                        op=mybir.AluOpType.subtract,
                    )
                    col = b * S + s
                    nc.scalar.activation(
                        out=scr_s[:], in_=diff_t[:],
                        func=mybir.ActivationFunctionType.Square,
                        accum_out=acc[:, col:col + 1],
                    )

                load_gt(0)
                load_gt(1)
                load_pred(0, 0)
                load_pred(0, 1)
                load_gt(2)
                load_pred(1, 0)
                load_pred(1, 1)
                load_gt(3)
                load_pred(2, 0)

                # ---- argmax stage (tiny ops; the row gather + lb gather
                # hide under the remaining pred streams) ----
                if mode != 'dma':
                    cm_p = psp.tile([NJ, P], F32, tag="cmp", space="PSUM")
                    nc.tensor.transpose(out=cm_p[:], in_=colmax[:], identity=id_t[:])
                    cmT = small.tile([NJ, P], F32, tag="cmT")
                    nc.vector.tensor_copy(out=cmT[:], in_=cm_p[:])
                    gmax = small.tile([NJ, 1], F32, tag="gmax")
                    nc.vector.tensor_reduce(out=gmax[:], in_=cmT[:],
                                            axis=mybir.AxisListType.X,
                                            op=mybir.AluOpType.max)
                    maskT = small.tile([NJ, P], F32, tag="maskT")
                    nc.vector.tensor_scalar(out=maskT[:], in0=cmT[:], scalar1=gmax[:],
                                            scalar2=None, op0=mybir.AluOpType.is_ge)
                    scoreT = small.tile([NJ, P], F32, tag="scoreT")
                    nc.vector.tensor_tensor(out=scoreT[:], in0=maskT[:], in1=negp_t[:],
                                            op=mybir.AluOpType.mult)
                    pscore = small.tile([NJ, 1], F32, tag="pscore")
                    nc.vector.tensor_reduce(out=pscore[:], in_=scoreT[:],
                                            axis=mybir.AxisListType.X,
                                            op=mybir.AluOpType.max)
                    # row units: off_row = rowbase - pscore*K
                    offr_f = small.tile([NJ, 1], F32, tag="offr_f")
                    nc.vector.scalar_tensor_tensor(
                        out=offr_f[:], in0=pscore[:], scalar=float(-K),
                        in1=rowb_t[:], op0=mybir.AluOpType.mult,
                        op1=mybir.AluOpType.add,
                    )
                    offr_i = small.tile([NJ, 1], I32, tag="offr_i")
                    nc.vector.tensor_copy(out=offr_i[:], in_=offr_f[:])
                    rows = small.tile([NJ, FK], F32, tag="rows")
                    nc.gpsimd.indirect_dma_start(
                        out=rows[:], out_offset=None, in_=gt,
                        in_offset=IndirectOffsetOnAxis(ap=offr_i[:], axis=0),
                    )

                load_pred(2, 1)

                if mode != 'dma':
                    # rowsum = sum((row >= gmax) * (512 - f)) = 512 - f*
                    rmsk = small.tile([NJ, FK], F32, tag="rmsk")
                    rs44 = small.tile([NJ, 1], F32, tag="rs44")
                    nc.vector.scalar_tensor_tensor(
                        out=rmsk[:], in0=rows[:], scalar=gmax[:],
                        in1=negf_t[:], op0=mybir.AluOpType.is_ge,
                        op1=mybir.AluOpType.mult, accum_out=rs44[:],
                    )
                    # flat = (128-pscore)*512 + (512-rs44)
                    t1 = small.tile([NJ, 1], F32, tag="t1")
                    nc.vector.tensor_scalar(out=t1[:], in0=pscore[:], scalar1=-512.0,
                                            scalar2=None, op0=mybir.AluOpType.mult)
                    flatf = small.tile([NJ, 1], F32, tag="flatf")
                    nc.vector.scalar_tensor_tensor(
                        out=flatf[:], in0=t1[:], scalar=float(P * FK + FK),
                        in1=rs44[:], op0=mybir.AluOpType.add,
                        op1=mybir.AluOpType.subtract,
                    )
                    off_f = small.tile([NJ, NSC], F32, tag="off_f")
                    nc.vector.tensor_scalar(out=off_f[:], in0=base_t[:],
                                            scalar1=flatf[:], scalar2=None,
                                            op0=mybir.AluOpType.add)
                    off_i = small.tile([NJ, NSC], I32, tag="off_i")
                    nc.vector.tensor_copy(out=off_i[:], in_=off_f[:])
                    # one indirect DMA per (s,c) column: multi-offsets-per-
                    # partition in a single 2D gather mis-gather on HW
                    # (CoreSim accepts it; flat stays right, values wrong)
                    gath = small.tile([NJ, NSC], F32, tag="gath")
                    for sc in range(NSC):
                        nc.gpsimd.indirect_dma_start(
                            out=gath[:, sc:sc + 1], out_offset=None, in_=lb,
                            in_offset=IndirectOffsetOnAxis(
                                ap=off_i[:, sc:sc + 1], axis=0),
                        )

                load_pred(3, 0)
                load_pred(3, 1)

                # ---- label-loss tail + outputs ----
                if mode != 'dma':
                    nc.sync.dma_start(out=dbg_flat, in_=flatf[:])
                    nc.sync.dma_start(out=dbg_gath, in_=gath[:])
                    ldiff = small.tile([NJ, NSC], F32, tag="ldiff")
                    nc.vector.tensor_tensor(out=ldiff[:], in0=gath[:], in1=lab_t[:],
                                            op=mybir.AluOpType.subtract)
                    lsq = small.tile([NJ, NSC], F32, tag="lsq")
                    nc.scalar.activation(out=lsq[:], in_=ldiff[:],
                                         func=mybir.ActivationFunctionType.Square)
                    persum = small.tile([NJ, S], F32, tag="persum")
                    nc.vector.tensor_reduce(
                        out=persum[:],
                        in_=lsq[:].rearrange("j (s c) -> j s c", s=S),
                        axis=mybir.AxisListType.X, op=mybir.AluOpType.add)
                    lb_p = psp.tile([B_LOC, S], F32, tag="lbp", space="PSUM")
                    nc.tensor.matmul(out=lb_p[:], lhsT=blk_t[:], rhs=persum[:],
                                     start=True, stop=True)
                    lb_s = small.tile([B_LOC, S], F32, tag="lbs")
                    nc.scalar.activation(out=lb_s[:], in_=lb_p[:],
                                         func=mybir.ActivationFunctionType.Copy,
                                         scale=1.0 / (K * C))
                    nc.sync.dma_start(out=lb_out, in_=lb_s[:])

                hm_p = psp.tile([1, B_LOC * S], F32, tag="hmp", space="PSUM")
                if mode != 'dma':
                    nc.tensor.matmul(out=hm_p[:], lhsT=ones_t[:], rhs=acc[:],
                                     start=True, stop=True)
                hm_s = small.tile([1, B_LOC * S], F32, tag="hms")
                nc.scalar.activation(out=hm_s[:], in_=hm_p[:],
                                     func=mybir.ActivationFunctionType.Copy,
                                     scale=1.0 / (K * HW))
                nc.sync.dma_start(out=hm_out, in_=hm_s[:])

    nc.compile()
    return nc


def _get_nc():
    if "nc" not in _CACHE:
        _CACHE["nc"] = _build()
    return _CACHE["nc"]


def make_in_maps(combined_hm_preds, combined_lb_preds, heatmaps, labels):
    consts = _consts()
    in_maps = []
    for c in range(8):
        sl = slice(c * B_LOC, (c + 1) * B_LOC)
        lab = np.asarray(labels[sl], np.float32)  # [4, 11, 7]
        lab_bc = np.broadcast_to(
            lab[:, :, None, :], (B_LOC, K, S, C)).reshape(NJ, NSC)
        # pre-transpose so partition rows are contiguous DMA lines
        hm_r = np.asarray(combined_hm_preds[sl], np.float32).reshape(
            B_LOC, S, K, P, FK).transpose(0, 1, 3, 2, 4).reshape(
            B_LOC, S, P, KF).astype(HM_NP)
        gt_r = np.asarray(heatmaps[sl], np.float32).reshape(
            B_LOC, K, P, FK).transpose(0, 2, 1, 3).reshape(B_LOC * P * K, FK)
        m = {
            "hm": np.ascontiguousarray(hm_r),
            "gt": np.ascontiguousarray(gt_r),
            "lb": np.ascontiguousarray(
                np.asarray(combined_lb_preds[sl], np.float32).reshape(
                    B_LOC * S * C * HW, 1)),
            "labels_bc": np.ascontiguousarray(lab_bc),
        }
        m.update(consts)
        in_maps.append(m)
    return in_maps


def run(in_maps, trace=False, **kw):
    nc = _get_nc()
    return run_bass_kernel_spmd(nc, in_maps, list(range(8)), trace=trace, **kw)


def kernel(combined_hm_preds, combined_lb_preds, heatmaps, labels):
    in_maps = make_in_maps(combined_hm_preds, combined_lb_preds, heatmaps,
                           labels)
    res = run(in_maps).results
    combined = np.concatenate(
        [r["hm_out"].reshape(B_LOC, S) for r in res], axis=0)
    labels_loss = np.concatenate([r["lb_out"] for r in res], axis=0)
    return combined.astype(np.float32), labels_loss.astype(np.float32)
